# revision 1
# baseline (speedup 1.0000x reference)
"""Multi-head attention (B=2, N=2048, C=768, H=12) on 8 trn2 cores.

Sharding: core i handles batch b = i//4 and head-group g = i%4 (3 heads each).
Per-core pipeline:
  1. QKV^T projection from host-pre-transposed xT [C, N] (fp32r matmuls):
       qT, kT produced d-major [64, N] per head; v produced n-major [N, 64]
       per head, augmented with a ones column (softmax denominator trick).
  2. Scores computed transposed: S^T[k, q] = kT_h.T-slice @ qT_h, so softmax
       (exp via ScalarE) and attn@V need no on-chip transposes.
  3. attn@V with lhsT = [v | 1]: out rows 0:64 = unnormalized attn_out^T,
       row 64 = softmax denominators.
  4. Normalize (fp32): reciprocal of row 64, PE-broadcast to 64 partitions,
       DVE multiply.
  5. AllGather attn_out^T shards [192, N] -> [768, N] within groups
       [[0,1,2,3],[4,5,6,7]] (same batch).
  6. Output projection column-sharded by w_proj columns (per-core input
       shard), bias folded in as a K=1 matmul. Output is out^T [192, N];
       host concatenates + transposes.

Matmuls run in float32r (1 PE cycle/row vs 4 for fp32); the softmax
normalization chain stays fp32.
"""

import numpy as np

B, N, C, H, HD = 2, 2048, 768, 12, 64
G = 4              # tensor-parallel head groups
HL = H // G        # 3 heads per core
CHL = HL * HD      # 192 local channels
SCALE = HD ** -0.5
NCORES = 8
CT = C // 128      # 6 contraction chunks
NT = N // 128      # 16 n chunks
QW = 1024          # q window width
NWIN = N // QW     # 2 windows
KT = N // 128      # 16 k chunks
FW = 512           # matmul free width (psum bank)

_CACHE = {}


def _build_nc():
    import os
    import concourse.bass as bass
    import concourse.bacc as bacc
    import concourse.tile as tile
    import concourse.mybir as mybir

    F32 = mybir.dt.float32
    F32R = mybir.dt.float32r
    AF = mybir.ActivationFunctionType
    debug = bool(int(os.environ.get("KERNEL_DEBUG", "0")))

    nc = bacc.Bacc(num_devices=NCORES)
    xT_d = nc.declare_dram_parameter("xT", [C, N], F32R, isOutput=False)
    wq_d = nc.declare_dram_parameter("wq", [C, CHL], F32R, isOutput=False)
    wk_d = nc.declare_dram_parameter("wk", [C, CHL], F32R, isOutput=False)
    wv_d = nc.declare_dram_parameter("wv", [C, CHL], F32R, isOutput=False)
    wpz_d = nc.declare_dram_parameter("wpz", [NCORES * CHL, CHL], F32R,
                                      isOutput=False)
    bp_d = nc.declare_dram_parameter("bp", [1, CHL], F32R, isOutput=False)
    out_d = nc.declare_dram_parameter("out", [CHL, N], F32, isOutput=True)
    if debug:
        dbg_sums = nc.declare_dram_parameter("dbg_sums", [HL * NWIN, QW], F32,
                                             isOutput=True)
        dbg_recip = nc.declare_dram_parameter("dbg_recip", [HL * NWIN, QW], F32,
                                              isOutput=True)

    with tile.TileContext(nc) as tc:
        with tc.tile_pool(name="dram", bufs=1, space="DRAM") as dram:
            ag_ins = [dram.tile([CHL, QW], F32R, name=f"ag_in{w}")
                      for w in range(NWIN)]
            ag_outs = [dram.tile([NCORES * CHL, QW], F32R, name=f"ag_out{w}",
                                 addr_space="Shared")
                       for w in range(NWIN)]

            with tc.tile_pool(name="persist", bufs=1) as P:
                # ---- inputs only needed through phase 1 (own pool) ----
                QIN = tc.alloc_tile_pool(name="qkv_in", bufs=1)
                xT_sb = QIN.tile([128, CT, N], F32R)
                for ct in range(CT):
                    nc.sync.dma_start(
                        out=xT_sb[:, ct, :],
                        in_=xT_d[ct * 128:(ct + 1) * 128, :],
                    )
                wq_sb = QIN.tile([128, CT, CHL], F32R)
                wk_sb = QIN.tile([128, CT, CHL], F32R)
                wv_sb = QIN.tile([128, CT, CHL], F32R)
                for wsb, wd in ((wq_sb, wq_d), (wk_sb, wk_d),
                                (wv_sb, wv_d)):
                    for ct in range(CT):
                        nc.sync.dma_start(
                            out=wsb[:, ct, :],
                            in_=wd[ct * 128:(ct + 1) * 128, :],
                        )
                KP = NCORES * CHL // 128   # 12 K-chunks over gathered rows
                wp_sb = P.tile([128, KP, CHL], F32R)
                for kp in range(KP):
                    nc.sync.dma_start(
                        out=wp_sb[:, kp, :],
                        in_=wpz_d[kp * 128:(kp + 1) * 128, :],
                    )
                bp_sb = P.tile([1, CHL], F32R)
                nc.sync.dma_start(out=bp_sb[:], in_=bp_d[:, :])

                ones_sb = P.tile([128, 64], F32)
                nc.vector.memset(ones_sb[:], 1.0)
                ones_q = P.tile([1, FW], F32R)
                nc.vector.memset(ones_q[:].bitcast(F32), 1.0)

                # ---- persistent QKV results ----
                q01_sb = P.tile([128, N], F32R)   # qT heads 0,1
                q2_sb = P.tile([64, N], F32R)     # qT head 2
                k01_sb = P.tile([128, N], F32R)
                k2_sb = P.tile([64, N], F32R)
                # [n, nt, h, 128]: col 0 = ones (sums), 1:64 zero, 64:128 = v
                v_sb = P.tile([128, NT, HL, 2 * HD], F32R)
                nc.vector.memset(v_sb[:, :, :, 0:1].bitcast(F32), 1.0)
                nc.vector.memset(v_sb[:, :, :, 1:HD].bitcast(F32), 0.0)

                # ---- phase 1: QKV projections ----
                with tc.tile_pool(name="qkv_ps", bufs=1, space="PSUM") as QP:
                    for dst, wsb, mlo, mhi in (
                        (q01_sb, wq_sb, 0, 128),
                        (q2_sb, wq_sb, 128, CHL),
                        (k01_sb, wk_sb, 0, 128),
                        (k2_sb, wk_sb, 128, CHL),
                    ):
                        m = mhi - mlo
                        for f in range(N // FW):
                            qk_ps = QP.tile([m, FW], F32, tag="qk", bufs=3,
                                            padded_shape=[128, FW])
                            for ct in range(CT):
                                nc.tensor.matmul(
                                    qk_ps[:],
                                    lhsT=wsb[:, ct, mlo:mhi],
                                    rhs=xT_sb[:, ct, f * FW:(f + 1) * FW],
                                    start=(ct == 0), stop=(ct == CT - 1),
                                )
                            nc.vector.tensor_copy(
                                dst[:, f * FW:(f + 1) * FW], qk_ps[:])
                    for nt in range(NT):
                        v_ps = QP.tile([128, CHL], F32, tag="v", bufs=2)
                        for ct in range(CT):
                            nc.tensor.matmul(
                                v_ps[:],
                                lhsT=xT_sb[:, ct, nt * 128:(nt + 1) * 128],
                                rhs=wv_sb[:, ct, :],
                                start=(ct == 0), stop=(ct == CT - 1),
                            )
                        nc.vector.tensor_copy(
                            v_sb[:, nt, :, HD:2 * HD],
                            v_ps[:].rearrange("p (h d) -> p h d", h=HL))
                QIN.release()

                # ---- phase 2: attention per (head, q-window) ----
                with tc.tile_pool(name="att_ps", bufs=1, space="PSUM") as AT, \
                        tc.tile_pool(name="att_sb", bufs=1) as AS:
                    for w in range(NWIN):
                        for h in range(HL):
                            qh = (q01_sb[0:64], q01_sb[64:128], q2_sb[0:64])[h]
                            kh = (k01_sb[0:64], k01_sb[64:128], k2_sb[0:64])[h]
                            q0 = w * QW
                            A = AT.tile([128, QW], F32, tag="A", bufs=1)
                            for kc in range(KT):
                                S = AT.tile([128, QW], F32, tag="S", bufs=3)
                                E = AS.tile([128, QW], F32R, tag="E", bufs=4)
                                for j in range(QW // FW):
                                    nc.tensor.matmul(
                                        S[:, j * FW:(j + 1) * FW],
                                        lhsT=kh[:, kc * 128:(kc + 1) * 128],
                                        rhs=qh[:, q0 + j * FW:q0 + (j + 1) * FW],
                                    )
                                nc.scalar.activation(E[:], S[:], AF.Exp,
                                                     scale=SCALE)
                                for j in range(QW // FW):
                                    nc.tensor.matmul(
                                        A[:, j * FW:(j + 1) * FW],
                                        lhsT=v_sb[:, kc, h, :],
                                        rhs=E[:, j * FW:(j + 1) * FW],
                                        start=(kc == 0), stop=(kc == KT - 1),
                                    )
                            # normalize: recip of row 0 (denominators),
                            # gpsimd-broadcast to all partitions, multiply.
                            R = AS.tile([1, QW], F32, tag="R", bufs=2)
                            bcs = AS.tile([128, QW], F32, tag="bcs", bufs=2)
                            attn_t = AS.tile([128, QW], F32R, tag="attn",
                                             bufs=3)
                            for j in range(QW // FW):
                                js = slice(j * FW, (j + 1) * FW)
                                nc.vector.reciprocal(R[0:1, js], A[0:1, js])
                                nc.gpsimd.partition_broadcast(
                                    bcs[:, js], R[0:1, js])
                                nc.vector.tensor_mul(attn_t[64:128, js],
                                                     A[64:128, js],
                                                     bcs[64:128, js])
                            nc.sync.dma_start(
                                out=ag_ins[w][h * HD:(h + 1) * HD, :],
                                in_=attn_t[64:128, :],
                            )
                            if debug:
                                dsum = AS.tile([65, QW], F32, tag="dsum",
                                               bufs=2)
                                nc.vector.tensor_copy(dsum[0:1, :],
                                                      A[0:1, :])
                                nc.sync.dma_start(
                                    out=dbg_sums[h * NWIN + w:h * NWIN + w + 1, :],
                                    in_=dsum[0:1, :])
                                nc.sync.dma_start(
                                    out=dbg_recip[h * NWIN + w:h * NWIN + w + 1, :],
                                    in_=R[0:1, :])
                        # per-window 8-core AllGather: window 0's gather
                        # overlaps window 1's attention compute
                        nc.gpsimd.collective_compute(
                            "AllGather",
                            mybir.AluOpType.bypass,
                            replica_groups=[list(range(NCORES))],
                            ins=[ag_ins[w].opt()],
                            outs=[ag_outs[w].opt()],
                        )

                # ---- phase 4: output projection (out^T [CHL, N]) ----
                with tc.tile_pool(name="proj_ps", bufs=1, space="PSUM") as PP, \
                        tc.tile_pool(name="proj_sb", bufs=1) as PS:
                    for f in range(N // FW):
                        wf, jf = divmod(f, QW // FW)
                        ao_ts = []
                        for kp in range(KP):
                            ao_t = PS.tile([128, FW], F32R, tag="ao",
                                           bufs=2 * KP)
                            nc.sync.dma_start(
                                out=ao_t[:],
                                in_=ag_outs[wf][kp * 128:(kp + 1) * 128,
                                                jf * FW:(jf + 1) * FW],
                            )
                            ao_ts.append(ao_t)
                        for mlo, mhi in ((0, 128), (128, CHL)):
                            m = mhi - mlo
                            pr_ps = PP.tile([m, FW], F32, tag="pr", bufs=4,
                                            padded_shape=[128, FW])
                            first = True
                            for kp in range(KP):
                                nc.tensor.matmul(
                                    pr_ps[:],
                                    lhsT=wp_sb[:, kp, mlo:mhi],
                                    rhs=ao_ts[kp][:],
                                    start=first, stop=False,
                                )
                                first = False
                            nc.tensor.matmul(
                                pr_ps[:],
                                lhsT=bp_sb[:, mlo:mhi],
                                rhs=ones_q[:],
                                start=False, stop=True,
                            )
                            o_t = PS.tile([m, FW], F32, tag="o", bufs=3,
                                          padded_shape=[128, FW])
                            nc.vector.tensor_copy(o_t[:], pr_ps[:])
                            nc.sync.dma_start(
                                out=out_d[mlo:mhi, f * FW:(f + 1) * FW],
                                in_=o_t[:],
                            )
    nc.finalize()
    return nc


def get_nc():
    if "nc" not in _CACHE:
        _CACHE["nc"] = _build_nc()
    return _CACHE["nc"]


def make_in_maps(x, w_qkv, w_proj, b_proj):
    x = np.asarray(x, dtype=np.float32)
    w_qkv = np.asarray(w_qkv, dtype=np.float32)
    w_proj = np.asarray(w_proj, dtype=np.float32)
    b_proj = np.asarray(b_proj, dtype=np.float32)
    in_maps = []
    for core in range(NCORES):
        b, g = divmod(core, G)
        cs = slice(g * CHL, (g + 1) * CHL)
        im = {
            "xT": np.ascontiguousarray(x[b].T),
            "wq": np.ascontiguousarray(w_qkv[:, 0 * C:1 * C][:, cs]),
            "wk": np.ascontiguousarray(w_qkv[:, 1 * C:2 * C][:, cs]),
            "wv": np.ascontiguousarray(w_qkv[:, 2 * C:3 * C][:, cs]),
            "bp": np.ascontiguousarray(b_proj[cs].reshape(1, CHL)),
        }
        wpz = np.zeros((NCORES * CHL, CHL), np.float32)
        for j in range(NCORES):
            if j // G == b:
                gj = j % G
                wpz[j * CHL:(j + 1) * CHL] = \
                    w_proj[gj * CHL:(gj + 1) * CHL, cs]
        im["wpz"] = wpz
        in_maps.append(im)
    return in_maps


def unshard(results):
    out = np.empty((B, N, C), dtype=np.float32)
    for b in range(B):
        outT = np.concatenate(
            [results[b * G + g]["out"] for g in range(G)], axis=0)
        out[b] = outT.T
    return out


def kernel(x, w_qkv, w_proj, b_proj):
    from concourse.bass_utils import run_bass_kernel_spmd

    nc = get_nc()
    in_maps = make_in_maps(x, w_qkv, w_proj, b_proj)
    res = run_bass_kernel_spmd(nc, in_maps, list(range(NCORES)))
    return unshard(res.results)



# revision 7
# speedup vs baseline: 1.4139x; 1.4139x over previous
"""Multi-head attention (B=2, N=2048, C=768, H=12) on 8 trn2 cores.

Sharding: core i handles batch b = i//4 and head-group g = i%4 (3 heads each).
All matmul operands are bf16 (host-converted); accumulation is fp32 in PSUM
and the softmax normalization chain is fp32.

Per-core pipeline (q processed in 4 windows of 512 columns):
  1. QKV projection from host-pre-transposed xT [C, N]: qT/kT d-major
     [64, N] per head, v n-major [N, 64] per head augmented with a ones
     column (softmax denominator trick).
  2. Scores transposed: S^T[k, q] = kT_h chunk.T @ qT_h — no on-chip
     transposes anywhere.  exp via ScalarE over kc-PAIRS ([128, 2, 512]
     PSUM tiles viewed as [128, 1024]) to amortize ACT per-call overhead.
  3. attn@V with lhsT = [1 | v]: row 0 = softmax denominators, rows 1:65 =
     unnormalized attn_out^T.
  4. Normalize: reciprocal_approx_fast (fast custom-DVE op) of row 0,
     gpsimd partition-broadcast, DVE multiply -> bf16.
  5. Per-window AllGather (bf16) within same-batch 4-core groups
     [[0,1,2,3],[4,5,6,7]] -> [768, 512] gathered attn^T.
  6. Output projection column-sharded by w_proj columns; bias folded in as
     a K=1 matmul.  Output out^T [192, N] fp32; host concatenates +
     transposes.

Emission is interleaved so PE/ACT stay dense: window 0's head-2 rounds are
sprinkled between QKV chunk groups (dep-exact), each later window runs
proj(w-1) in its head-2 phase, and h0/h1 rounds alternate to keep ACT fed.
"""

import numpy as np

B, N, C, H, HD = 2, 2048, 768, 12, 64
G = 4              # tensor-parallel head groups
HL = H // G        # 3 heads per core
CHL = HL * HD      # 192 local channels
SCALE = HD ** -0.5
NCORES = 8
CT = C // 128      # 6 contraction chunks
FW = 512           # matmul free width == q window width
NWIN = N // FW     # 4 q windows
KT = N // 128      # 16 k chunks (8 pairs)
NP = KT // 2       # 8 kc pairs per (head, window)
KP = G * CHL // 128  # 6 gathered-row chunks

_CACHE = {}


def _build_nc():
    import concourse.bass as bass  # noqa: F401
    import concourse.bacc as bacc
    import concourse.tile as tile
    import concourse.mybir as mybir

    F32 = mybir.dt.float32
    BF16 = mybir.dt.bfloat16
    AF = mybir.ActivationFunctionType

    nc = bacc.Bacc(num_devices=NCORES)
    xT_d = nc.declare_dram_parameter("xT", [C, N], BF16, isOutput=False)
    wq_d = nc.declare_dram_parameter("wq", [C, CHL], BF16, isOutput=False)
    wk_d = nc.declare_dram_parameter("wk", [C, CHL], BF16, isOutput=False)
    wv_d = nc.declare_dram_parameter("wv", [C, CHL], BF16, isOutput=False)
    wp_d = nc.declare_dram_parameter("wp", [G * CHL, CHL], BF16,
                                     isOutput=False)
    bp_d = nc.declare_dram_parameter("bp", [1, CHL], BF16, isOutput=False)
    out_d = nc.declare_dram_parameter("out", [CHL, N], F32, isOutput=True)

    with tile.TileContext(nc) as tc:
        with tc.tile_pool(name="dram", bufs=1, space="DRAM") as dram:
            ag_ins = [dram.tile([CHL, FW], BF16, name=f"ag_in{w}")
                      for w in range(NWIN)]
            ag_outs = [dram.tile([G * CHL, FW], BF16, name=f"ag_out{w}")
                       for w in range(NWIN)]

            with tc.tile_pool(name="sb", bufs=1) as P, \
                    tc.tile_pool(name="ps", bufs=1, space="PSUM") as PS, \
                    tc.tile_pool(name="asb", bufs=1) as AS:

                # ---- input DMAs: qk weights, then x column blocks ----
                wq_sb = P.tile([128, CT, CHL], BF16)
                wk_sb = P.tile([128, CT, CHL], BF16)
                wv_sb = P.tile([128, CT, CHL], BF16)
                for wsb, wd in ((wq_sb, wq_d), (wk_sb, wk_d), (wv_sb, wv_d)):
                    for ct in range(CT):
                        nc.sync.dma_start(
                            out=wsb[:, ct, :],
                            in_=wd[ct * 128:(ct + 1) * 128, :],
                        )
                xT_sb = P.tile([128, CT, N], BF16)
                for blk in range(NWIN):
                    for ct in range(CT):
                        nc.sync.dma_start(
                            out=xT_sb[:, ct, blk * FW:(blk + 1) * FW],
                            in_=xT_d[ct * 128:(ct + 1) * 128,
                                     blk * FW:(blk + 1) * FW],
                        )
                wp_sb = P.tile([128, KP, CHL], BF16)
                for kp in range(KP):
                    nc.sync.dma_start(
                        out=wp_sb[:, kp, :],
                        in_=wp_d[kp * 128:(kp + 1) * 128, :],
                    )
                bp_sb = P.tile([1, CHL], BF16)
                nc.sync.dma_start(out=bp_sb[:], in_=bp_d[:, :])
                ones_q = P.tile([1, FW], BF16)
                nc.vector.memset(ones_q[:], 1.0)

                # ---- persistent QKV results (bf16) ----
                q01 = P.tile([128, N], BF16)   # qT heads 0,1
                q2 = P.tile([64, N], BF16)     # qT head 2
                k01 = P.tile([128, N], BF16)
                k2 = P.tile([64, N], BF16)
                # [kpos, kc, h, 128]: col 0 = ones (denominators), 1:64 =
                # zeros, 64:128 = v.  (attn rows land at partition base 64,
                # which engine APs require; base 1 is illegal.)
                v_sb = P.tile([128, KT, HL, 2 * HD], BF16)
                nc.vector.memset(v_sb[:, :, :, 0:1], 1.0)
                nc.vector.memset(v_sb[:, :, :, 1:HD], 0.0)

                QH = (q01[0:64], q01[64:128], q2[0:64])
                KH = (k01[0:64], k01[64:128], k2[0:64])

                def emit_qk(f):
                    for dst, wsb, mlo, mhi in (
                        (q01, wq_sb, 0, 128),
                        (q2, wq_sb, 128, CHL),
                        (k01, wk_sb, 0, 128),
                        (k2, wk_sb, 128, CHL),
                    ):
                        m = mhi - mlo
                        ps_t = PS.tile([m, FW], F32, tag="mm", bufs=2,
                                       padded_shape=[128, FW])
                        for ct in range(CT):
                            nc.tensor.matmul(
                                ps_t[:],
                                lhsT=wsb[:, ct, mlo:mhi],
                                rhs=xT_sb[:, ct, f * FW:(f + 1) * FW],
                                start=(ct == 0), stop=(ct == CT - 1),
                            )
                        nc.vector.tensor_copy(
                            dst[:, f * FW:(f + 1) * FW], ps_t[:])

                def emit_v(nt):
                    ps_t = PS.tile([128, CHL], F32, tag="mm", bufs=2,
                                   padded_shape=[128, FW])
                    for ct in range(CT):
                        nc.tensor.matmul(
                            ps_t[:],
                            lhsT=xT_sb[:, ct, nt * 128:(nt + 1) * 128],
                            rhs=wv_sb[:, ct, :],
                            start=(ct == 0), stop=(ct == CT - 1),
                        )
                    nc.vector.tensor_copy(
                        v_sb[:, nt, :, HD:2 * HD],
                        ps_t[:].rearrange("p (h d) -> p h d", h=HL))

                def new_A():
                    return PS.tile([128, FW], F32, tag="A", bufs=2, name="A")

                def att_round(w, h, p, A):
                    S = PS.tile([128, 2, FW], F32, tag="S", bufs=2)
                    E = AS.tile([128, 2, FW], BF16, tag="E", bufs=4)
                    for j in range(2):
                        kc = 2 * p + j
                        nc.tensor.matmul(
                            S[:, j, :],
                            lhsT=KH[h][:, kc * 128:(kc + 1) * 128],
                            rhs=QH[h][:, w * FW:(w + 1) * FW],
                        )
                    nc.scalar.activation(E[:, :, :], S[:, :, :], AF.Exp,
                                         scale=SCALE)
                    for j in range(2):
                        kc = 2 * p + j
                        nc.tensor.matmul(
                            A[:],
                            lhsT=v_sb[:, kc, h, :],
                            rhs=E[:, j, :],
                            start=(p == 0 and j == 0),
                            stop=(p == NP - 1 and j == 1),
                        )

                def norm_store(w, h, A):
                    R = AS.tile([1, FW], F32, tag="R", bufs=2)
                    bcs = AS.tile([128, FW], F32, tag="bcs", bufs=2)
                    attn_t = AS.tile([128, FW], BF16, tag="attn", bufs=3)
                    nc.vector.reciprocal_approx_fast(out=R[:], in_=A[0:1, :])
                    nc.gpsimd.partition_broadcast(bcs[:], R[0:1, :])
                    nc.vector.tensor_mul(attn_t[64:128, :], A[64:128, :],
                                         bcs[64:128, :])
                    nc.sync.dma_start(
                        out=ag_ins[w][h * HD:(h + 1) * HD, :],
                        in_=attn_t[64:128, :])

                def emit_gather(w):
                    nc.gpsimd.collective_compute(
                        "AllGather",
                        mybir.AluOpType.bypass,
                        replica_groups=[[0, 1, 2, 3], [4, 5, 6, 7]],
                        ins=[ag_ins[w].opt()],
                        outs=[ag_outs[w].opt()],
                    )

                def emit_proj(w):
                    ao_t = AS.tile([128, KP, FW], BF16, tag="ao", bufs=2)
                    for kp in range(KP):
                        nc.sync.dma_start(
                            out=ao_t[:, kp, :],
                            in_=ag_outs[w][kp * 128:(kp + 1) * 128, :],
                        )
                    for mlo, mhi in ((0, 128), (128, CHL)):
                        m = mhi - mlo
                        pr = PS.tile([m, FW], F32, tag="mm", bufs=2,
                                     padded_shape=[128, FW])
                        for kp in range(KP):
                            nc.tensor.matmul(
                                pr[:],
                                lhsT=wp_sb[:, kp, mlo:mhi],
                                rhs=ao_t[:, kp, :],
                                start=(kp == 0), stop=False,
                            )
                        nc.tensor.matmul(
                            pr[:], lhsT=bp_sb[:, mlo:mhi], rhs=ones_q[:],
                            start=False, stop=True,
                        )
                        o_t = AS.tile([m, FW], F32, tag="o", bufs=2,
                                      padded_shape=[128, FW])
                        nc.vector.tensor_copy(o_t[:], pr[:])
                        nc.sync.dma_start(
                            out=out_d[mlo:mhi, w * FW:(w + 1) * FW],
                            in_=o_t[:],
                        )

                # ----------------- emission schedule -----------------
                # Window 0: head-2 rounds interleaved with QKV chunk groups.
                # Dep map per h2 pair p: scores need k2 f=(p//2); AV needs
                # v chunks 2p, 2p+1.
                A2 = new_A()
                emit_qk(0)
                emit_v(0); emit_v(1)
                att_round(0, 2, 0, A2)
                emit_qk(1)
                emit_v(2); emit_v(3)
                att_round(0, 2, 1, A2)
                emit_v(4); emit_v(5)
                att_round(0, 2, 2, A2)
                emit_qk(2)
                emit_v(6); emit_v(7)
                att_round(0, 2, 3, A2)
                emit_v(8); emit_v(9)
                att_round(0, 2, 4, A2)
                emit_qk(3)
                emit_v(10); emit_v(11)
                att_round(0, 2, 5, A2)
                emit_v(12); emit_v(13)
                att_round(0, 2, 6, A2)
                emit_v(14); emit_v(15)
                att_round(0, 2, 7, A2)
                norm_store(0, 2, A2)
                A0, A1 = new_A(), new_A()
                for p in range(NP):
                    att_round(0, 0, p, A0)
                    att_round(0, 1, p, A1)
                norm_store(0, 0, A0)
                norm_store(0, 1, A1)
                emit_gather(0)

                # Windows 1..3: h2 phase hosts proj(w-1); h0/h1 alternate.
                for w in range(1, NWIN):
                    A2 = new_A()
                    for p in range(NP):
                        att_round(w, 2, p, A2)
                        if p == 3:
                            emit_proj(w - 1)
                    norm_store(w, 2, A2)
                    A0, A1 = new_A(), new_A()
                    for p in range(NP):
                        att_round(w, 0, p, A0)
                        att_round(w, 1, p, A1)
                    norm_store(w, 0, A0)
                    norm_store(w, 1, A1)
                    emit_gather(w)
                emit_proj(NWIN - 1)
    nc.finalize()
    return nc


def get_nc():
    if "nc" not in _CACHE:
        _CACHE["nc"] = _build_nc()
    return _CACHE["nc"]


def make_in_maps(x, w_qkv, w_proj, b_proj):
    from ml_dtypes import bfloat16

    x = np.asarray(x, dtype=np.float32)
    w_qkv = np.asarray(w_qkv, dtype=np.float32)
    w_proj = np.asarray(w_proj, dtype=np.float32)
    b_proj = np.asarray(b_proj, dtype=np.float32)
    in_maps = []
    for core in range(NCORES):
        b, g = divmod(core, G)
        cs = slice(g * CHL, (g + 1) * CHL)
        im = {
            "xT": np.ascontiguousarray(x[b].T).astype(bfloat16),
            "wq": np.ascontiguousarray(
                w_qkv[:, 0 * C:1 * C][:, cs]).astype(bfloat16),
            "wk": np.ascontiguousarray(
                w_qkv[:, 1 * C:2 * C][:, cs]).astype(bfloat16),
            "wv": np.ascontiguousarray(
                w_qkv[:, 2 * C:3 * C][:, cs]).astype(bfloat16),
            "wp": np.ascontiguousarray(w_proj[:, cs]).astype(bfloat16),
            "bp": np.ascontiguousarray(
                b_proj[cs].reshape(1, CHL)).astype(bfloat16),
        }
        in_maps.append(im)
    return in_maps


def unshard(results):
    out = np.empty((B, N, C), dtype=np.float32)
    for b in range(B):
        outT = np.concatenate(
            [results[b * G + g]["out"] for g in range(G)], axis=0)
        out[b] = outT.T
    return out


def kernel(x, w_qkv, w_proj, b_proj):
    from concourse.bass_utils import run_bass_kernel_spmd

    nc = get_nc()
    in_maps = make_in_maps(x, w_qkv, w_proj, b_proj)
    res = run_bass_kernel_spmd(nc, in_maps, list(range(NCORES)))
    return unshard(res.results)


# revision 9
# speedup vs baseline: 1.6700x; 1.1812x over previous
"""Multi-head attention (B=2, N=2048, C=768, H=12) on 8 trn2 cores.

Sharding: core i handles batch b = i//4 and head-group g = i%4 (3 heads each).
All matmul operands are bf16 (host-converted); accumulation is fp32 in PSUM
and the softmax normalization chain is fp32.

Per-core pipeline (q processed in 4 windows of 512 columns):
  1. QKV projection from host-pre-packed xT blocks: qT/kT d-major [64, N]
     per head, v n-major [N, 64] per head stored as [1|0*63|v] (softmax
     denominator ones trick; attn rows land at partition base 64).
  2. Scores transposed: S^T[k, q] = kT_h chunk.T @ qT_h.  exp via ScalarE
     over kc-PAIRS ([128, 2, 512] PSUM tiles viewed as [128, 1024]) to
     amortize ACT per-call overhead.  Rounds are software-pipelined
     (scores for pair p+1 are emitted before attn@V of pair p) so ScalarE
     never starves; h0/h1 score matmuls are adjacent with partition bases
     0/64 so they run concurrently in distinct PE row-groups.
  3. attn@V accumulates [denom; 0; attn^T] in one PSUM bank per head.
  4. Normalize: reciprocal_approx_fast of row 0, gpsimd partition
     broadcast, DVE multiply -> bf16.
  5. Split per-window AllGathers (bf16, 4-core same-batch groups
     [[0,1,2,3],[4,5,6,7]]): gather_b carries head 2 right after it
     finishes (mid-window), gather_a carries heads 0/1 at window end.
  6. Output projection column-sharded by w_proj columns, K-accumulated in
     two parts (gathered-b rows, then gathered-a rows + bias as a K=1
     matmul) so the last window's tail is only gather_a + 4 matmuls.
     Output out^T [192, N] fp32; host concatenates + transposes.

Window 0's head-2 phase is interleaved with the QKV chunk groups
(dep-exact) so the PE has no phase boundary; later windows run proj parts
in their slack.  A tiny warm-up AllGather at the top absorbs the ~11us
TOPSP cold-start so the first real gather is fast.
"""

import numpy as np

B, N, C, H, HD = 2, 2048, 768, 12, 64
G = 4              # tensor-parallel head groups
HL = H // G        # 3 heads per core
CHL = HL * HD      # 192 local channels
SCALE = HD ** -0.5
NCORES = 8
CT = C // 128      # 6 contraction chunks
FW = 512           # matmul free width == q window width
NWIN = N // FW     # 4 q windows
KT = N // 128      # 16 k chunks (8 pairs)
NP = KT // 2       # 8 kc pairs per (head, window)
KPA = 4            # gathered-a row chunks (heads 0/1: 4*128)
KPB = 2            # gathered-b row chunks (head 2: 4*64)

_CACHE = {}


def _build_nc():
    import concourse.bass as bass  # noqa: F401
    import concourse.bacc as bacc
    import concourse.tile as tile
    import concourse.mybir as mybir

    F32 = mybir.dt.float32
    BF16 = mybir.dt.bfloat16
    AF = mybir.ActivationFunctionType

    nc = bacc.Bacc(num_devices=NCORES)
    # All inputs host-packed so each loads with a single (or per-block) DMA.
    xT_d = nc.declare_dram_parameter("xT", [128, CT, N], BF16, isOutput=False)
    wq_d = nc.declare_dram_parameter("wq", [128, CT, CHL], BF16,
                                     isOutput=False)
    wk_d = nc.declare_dram_parameter("wk", [128, CT, CHL], BF16,
                                     isOutput=False)
    wv_d = nc.declare_dram_parameter("wv", [128, CT, CHL], BF16,
                                     isOutput=False)
    wpa_d = nc.declare_dram_parameter("wpa", [128, KPA, CHL], BF16,
                                      isOutput=False)
    wpb_d = nc.declare_dram_parameter("wpb", [128, KPB, CHL], BF16,
                                      isOutput=False)
    bp_d = nc.declare_dram_parameter("bp", [1, CHL], BF16, isOutput=False)
    out_d = nc.declare_dram_parameter("out", [CHL, N], F32, isOutput=True)

    RG = [[0, 1, 2, 3], [4, 5, 6, 7]]

    with tile.TileContext(nc) as tc:
        with tc.tile_pool(name="dram", bufs=1, space="DRAM") as dram:
            ag_in_a = [dram.tile([128, FW], BF16, name=f"ag_in_a{w}")
                       for w in range(NWIN)]
            ag_out_a = [dram.tile([G * 128, FW], BF16, name=f"ag_out_a{w}")
                        for w in range(NWIN)]
            ag_in_b = [dram.tile([64, FW], BF16, name=f"ag_in_b{w}")
                       for w in range(NWIN)]
            ag_out_b = [dram.tile([G * 64, FW], BF16, name=f"ag_out_b{w}")
                        for w in range(NWIN)]
            warm_in = dram.tile([1, 64], BF16, name="warm_in")
            warm_out = dram.tile([G, 64], BF16, name="warm_out")

            with tc.tile_pool(name="sb", bufs=1) as P, \
                    tc.tile_pool(name="ps", bufs=1, space="PSUM") as PS, \
                    tc.tile_pool(name="asb", bufs=1) as AS:

                # ---- input DMAs (one per tensor / xT block) ----
                wq_sb = P.tile([128, CT, CHL], BF16)
                wk_sb = P.tile([128, CT, CHL], BF16)
                wv_sb = P.tile([128, CT, CHL], BF16)
                nc.sync.dma_start(out=wk_sb[:], in_=wk_d[:, :, :])
                nc.sync.dma_start(out=wq_sb[:], in_=wq_d[:, :, :])
                # TOPSP warm-up: tiny gather, runs under the input DMAs.
                nc.gpsimd.collective_compute(
                    "AllGather", mybir.AluOpType.bypass, replica_groups=RG,
                    ins=[warm_in.opt()], outs=[warm_out.opt()])
                xT_sb = P.tile([128, CT, N], BF16)
                for blk in range(NWIN):
                    nc.sync.dma_start(
                        out=xT_sb[:, :, blk * FW:(blk + 1) * FW],
                        in_=xT_d[:, :, blk * FW:(blk + 1) * FW],
                    )
                nc.sync.dma_start(out=wv_sb[:], in_=wv_d[:, :, :])
                wpa_sb = P.tile([128, KPA, CHL], BF16)
                wpb_sb = P.tile([128, KPB, CHL], BF16)
                nc.sync.dma_start(out=wpa_sb[:], in_=wpa_d[:, :, :])
                nc.sync.dma_start(out=wpb_sb[:], in_=wpb_d[:, :, :])
                bp_sb = P.tile([1, CHL], BF16)
                nc.sync.dma_start(out=bp_sb[:], in_=bp_d[:, :])
                ones_q = P.tile([1, FW], BF16)
                nc.vector.memset(ones_q[:], 1.0)

                # ---- persistent QKV results (bf16) ----
                q01 = P.tile([128, N], BF16)   # qT heads 0,1
                q2 = P.tile([64, N], BF16)     # qT head 2
                k01 = P.tile([128, N], BF16)
                k2 = P.tile([64, N], BF16)
                # [kpos, kc, h, 128]: col 0 = ones, 1:64 zeros, 64:128 = v
                v_sb = P.tile([128, KT, HL, 2 * HD], BF16)
                nc.vector.memset(v_sb[:, :, :, 0:1], 1.0)
                nc.vector.memset(v_sb[:, :, :, 1:HD], 0.0)

                QH = (q01[0:64], q01[64:128], q2[0:64])
                KH = (k01[0:64], k01[64:128], k2[0:64])

                def emit_qk(f):
                    for dst, wsb, mlo, mhi in (
                        (k2, wk_sb, 128, CHL),
                        (q2, wq_sb, 128, CHL),
                        (k01, wk_sb, 0, 128),
                        (q01, wq_sb, 0, 128),
                    ):
                        m = mhi - mlo
                        ps_t = PS.tile([m, FW], F32, tag="mm", bufs=2,
                                       padded_shape=[128, FW], name="qk_ps")
                        for ct in range(CT):
                            nc.tensor.matmul(
                                ps_t[:],
                                lhsT=wsb[:, ct, mlo:mhi],
                                rhs=xT_sb[:, ct, f * FW:(f + 1) * FW],
                                start=(ct == 0), stop=(ct == CT - 1),
                            )
                        nc.vector.tensor_copy(
                            dst[:, f * FW:(f + 1) * FW], ps_t[:])

                def emit_v(nt):
                    ps_t = PS.tile([128, CHL], F32, tag="mm", bufs=2,
                                   padded_shape=[128, FW], name="v_ps")
                    for ct in range(CT):
                        nc.tensor.matmul(
                            ps_t[:],
                            lhsT=xT_sb[:, ct, nt * 128:(nt + 1) * 128],
                            rhs=wv_sb[:, ct, :],
                            start=(ct == 0), stop=(ct == CT - 1),
                        )
                    nc.vector.tensor_copy(
                        v_sb[:, nt, :, HD:2 * HD],
                        ps_t[:].rearrange("p (h d) -> p h d", h=HL))

                def new_A():
                    return PS.tile([128, FW], F32, tag="A", bufs=2, name="A")

                def score_mms(w, hs, p):
                    """One S tile per head in hs; h0/h1 MMs adjacent per j
                    so their distinct PE row-groups run concurrently."""
                    Ss = {h: PS.tile([128, 2, FW], F32, tag="S", bufs=2,
                                     name="S") for h in hs}
                    for j in range(2):
                        kc = 2 * p + j
                        for h in hs:
                            nc.tensor.matmul(
                                Ss[h][:, j, :],
                                lhsT=KH[h][:, kc * 128:(kc + 1) * 128],
                                rhs=QH[h][:, w * FW:(w + 1) * FW],
                            )
                    return Ss

                def exp_mms(Ss, hs):
                    Es = {}
                    for h in hs:
                        E = AS.tile([128, 2, FW], BF16, tag="E", bufs=4,
                                    name="E")
                        nc.scalar.activation(E[:, :, :], Ss[h][:, :, :],
                                             AF.Exp, scale=SCALE)
                        Es[h] = E
                    return Es

                def av_mms(hs, p, Es, As):
                    for h in hs:
                        for j in range(2):
                            kc = 2 * p + j
                            nc.tensor.matmul(
                                As[h][:],
                                lhsT=v_sb[:, kc, h, :],
                                rhs=Es[h][:, j, :],
                                start=(p == 0 and j == 0),
                                stop=(p == NP - 1 and j == 1),
                            )

                def att_phase(w, hs, As, fillers=None):
                    """Software-pipelined rounds: scores for pair p+1 are
                    emitted before attn@V of pair p.  fillers[p] (optional)
                    emits extra PE work after round p's AVs."""
                    Ss = score_mms(w, hs, 0)
                    Es = exp_mms(Ss, hs)
                    for p in range(NP):
                        if p + 1 < NP:
                            Sn = score_mms(w, hs, p + 1)
                            En = exp_mms(Sn, hs)
                        av_mms(hs, p, Es, As)
                        if fillers is not None and p in fillers:
                            fillers[p]()
                        if p + 1 < NP:
                            Es = En
                    for h in hs:
                        norm_store(w, h, As[h])

                def norm_store(w, h, A):
                    R = AS.tile([1, FW], F32, tag="R", bufs=2, name="R")
                    bcs = AS.tile([128, FW], F32, tag="bcs", bufs=2,
                                  name="bcs")
                    attn_t = AS.tile([128, FW], BF16, tag="attn", bufs=3,
                                     name="attn_t")
                    nc.vector.reciprocal_approx_fast(out=R[:], in_=A[0:1, :])
                    nc.gpsimd.partition_broadcast(bcs[:], R[0:1, :])
                    nc.vector.tensor_mul(attn_t[64:128, :], A[64:128, :],
                                         bcs[64:128, :])
                    if h == 2:
                        nc.sync.dma_start(out=ag_in_b[w][:, :],
                                          in_=attn_t[64:128, :])
                    else:
                        nc.sync.dma_start(
                            out=ag_in_a[w][h * HD:(h + 1) * HD, :],
                            in_=attn_t[64:128, :])

                def emit_gather(w, which):
                    i, o = ((ag_in_a, ag_out_a) if which == "a"
                            else (ag_in_b, ag_out_b))
                    nc.gpsimd.collective_compute(
                        "AllGather", mybir.AluOpType.bypass,
                        replica_groups=RG,
                        ins=[i[w].opt()], outs=[o[w].opt()])

                proj_pr = {}

                def emit_proj_b(w):
                    """First half of proj(w): K-accumulate over gathered
                    head-2 rows.  pr tiles stay open for emit_proj_a."""
                    ao_b = AS.tile([128, KPB, FW], BF16, tag="aob", bufs=2,
                                   name="ao_b")
                    for kp in range(KPB):
                        nc.sync.dma_start(
                            out=ao_b[:, kp, :],
                            in_=ag_out_b[w][kp * 128:(kp + 1) * 128, :])
                    prs = []
                    for mlo, mhi in ((0, 128), (128, CHL)):
                        pr = PS.tile([mhi - mlo, FW], F32, tag="mm", bufs=2,
                                     padded_shape=[128, FW], name="pr")
                        for kp in range(KPB):
                            nc.tensor.matmul(
                                pr[:], lhsT=wpb_sb[:, kp, mlo:mhi],
                                rhs=ao_b[:, kp, :],
                                start=(kp == 0), stop=False)
                        prs.append(pr)
                    proj_pr[w] = prs

                def emit_proj_a(w):
                    ao_a = AS.tile([128, KPA, FW], BF16, tag="aoa", bufs=2,
                                   name="ao_a")
                    for kp in range(KPA):
                        nc.sync.dma_start(
                            out=ao_a[:, kp, :],
                            in_=ag_out_a[w][kp * 128:(kp + 1) * 128, :])
                    for mi, (mlo, mhi) in enumerate(((0, 128), (128, CHL))):
                        pr = proj_pr[w][mi]
                        for kp in range(KPA):
                            nc.tensor.matmul(
                                pr[:], lhsT=wpa_sb[:, kp, mlo:mhi],
                                rhs=ao_a[:, kp, :],
                                start=False, stop=False)
                        nc.tensor.matmul(
                            pr[:], lhsT=bp_sb[:, mlo:mhi], rhs=ones_q[:],
                            start=False, stop=True)
                        o_t = AS.tile([mhi - mlo, FW], F32, tag="o", bufs=2,
                                      padded_shape=[128, FW], name="o_t")
                        nc.vector.tensor_copy(o_t[:], pr[:])
                        nc.sync.dma_start(
                            out=out_d[mlo:mhi, w * FW:(w + 1) * FW],
                            in_=o_t[:])

                # ----------------- emission schedule -----------------
                # Window 0 head-2 phase interleaved with QKV (dep-exact:
                # pair p needs k2/q2 block <= p//2 and v chunks 2p, 2p+1;
                # simple non-pipelined rounds — QKV fills the exp waits).
                A2 = new_A()
                emit_qk(0)
                emit_v(0); emit_v(1)

                def w0_round(p):
                    Ss = score_mms(0, [2], p)
                    Es = exp_mms(Ss, [2])
                    av_mms([2], p, Es, {2: A2})

                w0_round(0)
                emit_qk(1)
                emit_v(2); emit_v(3)
                w0_round(1)
                emit_v(4); emit_v(5)
                w0_round(2)
                emit_qk(2)
                emit_v(6); emit_v(7)
                w0_round(3)
                emit_v(8); emit_v(9)
                w0_round(4)
                emit_qk(3)
                emit_v(10); emit_v(11)
                w0_round(5)
                emit_v(12); emit_v(13)
                w0_round(6)
                emit_v(14); emit_v(15)
                w0_round(7)
                norm_store(0, 2, A2)
                emit_gather(0, "b")
                A0, A1 = new_A(), new_A()
                att_phase(0, [0, 1], {0: A0, 1: A1},
                          fillers={5: lambda: emit_proj_b(0)})
                emit_gather(0, "a")

                for w in range(1, NWIN):
                    A2 = new_A()
                    att_phase(w, [2], {2: A2})
                    emit_gather(w, "b")
                    A0, A1 = new_A(), new_A()
                    att_phase(w, [0, 1], {0: A0, 1: A1},
                              fillers={2: (lambda wp=w - 1: emit_proj_a(wp)),
                                       5: (lambda wp=w: emit_proj_b(wp))})
                    emit_gather(w, "a")
                emit_proj_a(NWIN - 1)
    nc.finalize()
    return nc


def get_nc():
    if "nc" not in _CACHE:
        _CACHE["nc"] = _build_nc()
    return _CACHE["nc"]


def _pack128(a):
    """[K, M] -> [128, K//128, M] partition-major packing."""
    from ml_dtypes import bfloat16

    k, m = a.shape
    return np.ascontiguousarray(
        a.reshape(k // 128, 128, m).transpose(1, 0, 2)).astype(bfloat16)


def make_in_maps(x, w_qkv, w_proj, b_proj):
    from ml_dtypes import bfloat16

    x = np.asarray(x, dtype=np.float32)
    w_qkv = np.asarray(w_qkv, dtype=np.float32)
    w_proj = np.asarray(w_proj, dtype=np.float32)
    b_proj = np.asarray(b_proj, dtype=np.float32)
    # gathered-a rows: per rank r, w_proj rows [192r, 192r+128) (heads 0/1)
    # gathered-b rows: per rank r, w_proj rows [192r+128, 192(r+1)) (head 2)
    idx_a = np.concatenate(
        [np.arange(192 * r, 192 * r + 128) for r in range(G)])
    idx_b = np.concatenate(
        [np.arange(192 * r + 128, 192 * (r + 1)) for r in range(G)])
    in_maps = []
    for core in range(NCORES):
        b, g = divmod(core, G)
        cs = slice(g * CHL, (g + 1) * CHL)
        im = {
            "xT": _pack128(np.ascontiguousarray(x[b].T)),
            "wq": _pack128(np.ascontiguousarray(w_qkv[:, 0 * C:1 * C][:, cs])),
            "wk": _pack128(np.ascontiguousarray(w_qkv[:, 1 * C:2 * C][:, cs])),
            "wv": _pack128(np.ascontiguousarray(w_qkv[:, 2 * C:3 * C][:, cs])),
            "wpa": _pack128(np.ascontiguousarray(w_proj[idx_a][:, cs])),
            "wpb": _pack128(np.ascontiguousarray(w_proj[idx_b][:, cs])),
            "bp": np.ascontiguousarray(
                b_proj[cs].reshape(1, CHL)).astype(bfloat16),
        }
        in_maps.append(im)
    return in_maps


def unshard(results):
    out = np.empty((B, N, C), dtype=np.float32)
    for b in range(B):
        outT = np.concatenate(
            [results[b * G + g]["out"] for g in range(G)], axis=0)
        out[b] = outT.T
    return out


def kernel(x, w_qkv, w_proj, b_proj):
    from concourse.bass_utils import run_bass_kernel_spmd

    nc = get_nc()
    in_maps = make_in_maps(x, w_qkv, w_proj, b_proj)
    res = run_bass_kernel_spmd(nc, in_maps, list(range(NCORES)))
    return unshard(res.results)


# revision 10
# speedup vs baseline: 1.7164x; 1.0278x over previous
"""Multi-head attention (B=2, N=2048, C=768, H=12) on 8 trn2 cores.

Sharding: core i handles batch b = i//4 and head-group g = i%4 (3 heads each).
All matmul operands are bf16 (host-converted); accumulation is fp32 in PSUM
and the softmax normalization chain is fp32.

Per-core pipeline (q processed in 4 windows of 512 columns):
  1. QKV projection: qT/kT d-major [64, N] per head, v n-major [N, 64] per
     head stored as [1|0*63|v] (softmax denominator ones trick; attn rows
     land at partition base 64, which engine APs require).
  2. Scores transposed: S^T[k, q] = kT_h chunk.T @ qT_h.  exp via ScalarE
     over kc-PAIRS ([128, 2, 512] PSUM tiles viewed as [128, 1024]) to
     amortize ACT per-call overhead.  Rounds are software-pipelined
     (scores for pair p+1 emitted before attn@V of pair p) so ScalarE
     never starves; paired heads' score matmuls are adjacent with
     partition bases 0/64 so they run in distinct PE row-groups.
  3. attn@V accumulates [denom; 0; attn^T] in one PSUM bank per head.
  4. Normalize right after each head's phase: reciprocal_approx_fast of
     row 0, gpsimd partition broadcast, DVE multiply -> bf16 -> DRAM.
  5. ONE AllGather per window (bf16, 4-core same-batch groups
     [[0,1,2,3],[4,5,6,7]]), triggered at window end so its ~13us
     gpsimd completion-wait lands in the next window's slack.  Window 3
     instead runs three sequential head phases with three small 64-row
     gathers so the kernel tail is only one small gather + 2 matmuls.
  6. Output projection column-sharded by w_proj columns; proj(w) runs as
     PE fillers inside window w+1's rounds; bias folded in as a K=1
     matmul.  Output out^T [192, N] fp32; host concatenates + transposes.

Window 0's h2+h1 phases are interleaved with the QKV chunk groups
(dep-exact) so the PE has no phase boundary and ScalarE gets work early.
"""

import numpy as np

B, N, C, H, HD = 2, 2048, 768, 12, 64
G = 4              # tensor-parallel head groups
HL = H // G        # 3 heads per core
CHL = HL * HD      # 192 local channels
SCALE = HD ** -0.5
NCORES = 8
CT = C // 128      # 6 contraction chunks
FW = 512           # matmul free width == q window width
NWIN = N // FW     # 4 q windows
KT = N // 128      # 16 k chunks (8 pairs)
NP = KT // 2       # 8 kc pairs per (head, window)
KP = G * CHL // 128  # 6 gathered-row chunks (full window gather)
LW = NWIN - 1      # last window (split per-head gathers)

_CACHE = {}


def _build_nc():
    import concourse.bass as bass  # noqa: F401
    import concourse.bacc as bacc
    import concourse.tile as tile
    import concourse.mybir as mybir

    F32 = mybir.dt.float32
    BF16 = mybir.dt.bfloat16
    AF = mybir.ActivationFunctionType

    nc = bacc.Bacc(num_devices=NCORES)
    # Inputs host-packed so each loads with a single (or per-block) DMA.
    xT_d = nc.declare_dram_parameter("xT", [128, CT, N], BF16, isOutput=False)
    wq_d = nc.declare_dram_parameter("wq", [128, CT, CHL], BF16,
                                     isOutput=False)
    wk_d = nc.declare_dram_parameter("wk", [128, CT, CHL], BF16,
                                     isOutput=False)
    wv_d = nc.declare_dram_parameter("wv", [128, CT, CHL], BF16,
                                     isOutput=False)
    wpz_d = nc.declare_dram_parameter("wpz", [128, KP, CHL], BF16,
                                      isOutput=False)
    wps_d = nc.declare_dram_parameter("wps", [128, HL, 2, CHL], BF16,
                                      isOutput=False)
    bp_d = nc.declare_dram_parameter("bp", [1, CHL], BF16, isOutput=False)
    out_d = nc.declare_dram_parameter("out", [CHL, N], F32, isOutput=True)

    RG = [[0, 1, 2, 3], [4, 5, 6, 7]]

    with tile.TileContext(nc) as tc:
        with tc.tile_pool(name="dram", bufs=1, space="DRAM") as dram:
            ag_in = [dram.tile([CHL, FW], BF16, name=f"ag_in{w}")
                     for w in range(LW)]
            ag_out = [dram.tile([G * CHL, FW], BF16, name=f"ag_out{w}")
                      for w in range(LW)]
            ag3_in = [dram.tile([64, FW], BF16, name=f"ag3_in{h}")
                      for h in range(HL)]
            ag3_out = [dram.tile([G * 64, FW], BF16, name=f"ag3_out{h}")
                       for h in range(HL)]

            with tc.tile_pool(name="sb", bufs=1) as P, \
                    tc.tile_pool(name="ps", bufs=1, space="PSUM") as PS, \
                    tc.tile_pool(name="asb", bufs=1) as AS:

                # ---- input DMAs (one per tensor / xT block) ----
                wq_sb = P.tile([128, CT, CHL], BF16)
                wk_sb = P.tile([128, CT, CHL], BF16)
                wv_sb = P.tile([128, CT, CHL], BF16)
                nc.sync.dma_start(out=wk_sb[:], in_=wk_d[:, :, :])
                nc.sync.dma_start(out=wq_sb[:], in_=wq_d[:, :, :])
                xT_sb = P.tile([128, CT, N], BF16)
                for blk in range(NWIN):
                    nc.sync.dma_start(
                        out=xT_sb[:, :, blk * FW:(blk + 1) * FW],
                        in_=xT_d[:, :, blk * FW:(blk + 1) * FW],
                    )
                nc.sync.dma_start(out=wv_sb[:], in_=wv_d[:, :, :])
                wpz_sb = P.tile([128, KP, CHL], BF16)
                wps_sb = P.tile([128, HL, 2, CHL], BF16)
                nc.sync.dma_start(out=wpz_sb[:], in_=wpz_d[:, :, :])
                nc.sync.dma_start(out=wps_sb[:], in_=wps_d[:, :, :, :])
                bp_sb = P.tile([1, CHL], BF16)
                nc.sync.dma_start(out=bp_sb[:], in_=bp_d[:, :])
                ones_q = P.tile([1, FW], BF16)
                nc.vector.memset(ones_q[:], 1.0)

                # ---- persistent QKV results (bf16) ----
                q01 = P.tile([128, N], BF16)   # qT heads 0,1
                q2 = P.tile([64, N], BF16)     # qT head 2
                k01 = P.tile([128, N], BF16)
                k2 = P.tile([64, N], BF16)
                # [kpos, kc, h, 128]: col 0 = ones, 1:64 zeros, 64:128 = v
                v_sb = P.tile([128, KT, HL, 2 * HD], BF16)
                nc.vector.memset(v_sb[:, :, :, 0:1], 1.0)
                nc.vector.memset(v_sb[:, :, :, 1:HD], 0.0)

                QH = (q01[0:64], q01[64:128], q2[0:64])
                KH = (k01[0:64], k01[64:128], k2[0:64])

                def emit_qk(f):
                    for dst, wsb, mlo, mhi in (
                        (k2, wk_sb, 128, CHL),
                        (q2, wq_sb, 128, CHL),
                        (k01, wk_sb, 0, 128),
                        (q01, wq_sb, 0, 128),
                    ):
                        m = mhi - mlo
                        ps_t = PS.tile([m, FW], F32, tag="mm", bufs=2,
                                       padded_shape=[128, FW], name="qk_ps")
                        for ct in range(CT):
                            nc.tensor.matmul(
                                ps_t[:],
                                lhsT=wsb[:, ct, mlo:mhi],
                                rhs=xT_sb[:, ct, f * FW:(f + 1) * FW],
                                start=(ct == 0), stop=(ct == CT - 1),
                            )
                        nc.vector.tensor_copy(
                            dst[:, f * FW:(f + 1) * FW], ps_t[:])

                def emit_v(nt):
                    ps_t = PS.tile([128, CHL], F32, tag="mm", bufs=2,
                                   padded_shape=[128, FW], name="v_ps")
                    for ct in range(CT):
                        nc.tensor.matmul(
                            ps_t[:],
                            lhsT=xT_sb[:, ct, nt * 128:(nt + 1) * 128],
                            rhs=wv_sb[:, ct, :],
                            start=(ct == 0), stop=(ct == CT - 1),
                        )
                    nc.vector.tensor_copy(
                        v_sb[:, nt, :, HD:2 * HD],
                        ps_t[:].rearrange("p (h d) -> p h d", h=HL))

                def new_A():
                    return PS.tile([128, FW], F32, tag="A", bufs=2, name="A")

                def score_mms(w, hs, p):
                    Ss = {h: PS.tile([128, 2, FW], F32, tag="S", bufs=2,
                                     name="S") for h in hs}
                    for j in range(2):
                        kc = 2 * p + j
                        for h in hs:
                            nc.tensor.matmul(
                                Ss[h][:, j, :],
                                lhsT=KH[h][:, kc * 128:(kc + 1) * 128],
                                rhs=QH[h][:, w * FW:(w + 1) * FW],
                            )
                    return Ss

                def exp_mms(Ss, hs):
                    Es = {}
                    for h in hs:
                        E = AS.tile([128, 2, FW], BF16, tag="E", bufs=4,
                                    name="E")
                        nc.scalar.activation(E[:, :, :], Ss[h][:, :, :],
                                             AF.Exp, scale=SCALE)
                        Es[h] = E
                    return Es

                def av_mms(hs, p, Es, As):
                    for h in hs:
                        for j in range(2):
                            kc = 2 * p + j
                            nc.tensor.matmul(
                                As[h][:],
                                lhsT=v_sb[:, kc, h, :],
                                rhs=Es[h][:, j, :],
                                start=(p == 0 and j == 0),
                                stop=(p == NP - 1 and j == 1),
                            )

                def norm_store(w, h, A):
                    R = AS.tile([1, FW], F32, tag="R", bufs=2, name="R")
                    bcs = AS.tile([128, FW], F32, tag="bcs", bufs=2,
                                  name="bcs")
                    attn_t = AS.tile([128, FW], BF16, tag="attn", bufs=3,
                                     name="attn_t")
                    nc.vector.reciprocal_approx_fast(out=R[:], in_=A[0:1, :])
                    nc.gpsimd.partition_broadcast(bcs[:], R[0:1, :])
                    nc.vector.tensor_mul(attn_t[64:128, :], A[64:128, :],
                                         bcs[64:128, :])
                    if w == LW:
                        nc.sync.dma_start(out=ag3_in[h][:, :],
                                          in_=attn_t[64:128, :])
                    else:
                        nc.sync.dma_start(
                            out=ag_in[w][h * HD:(h + 1) * HD, :],
                            in_=attn_t[64:128, :])

                def emit_gather(w):
                    nc.gpsimd.collective_compute(
                        "AllGather", mybir.AluOpType.bypass,
                        replica_groups=RG,
                        ins=[ag_in[w].opt()], outs=[ag_out[w].opt()])

                def emit_gather3(h):
                    nc.gpsimd.collective_compute(
                        "AllGather", mybir.AluOpType.bypass,
                        replica_groups=RG,
                        ins=[ag3_in[h].opt()], outs=[ag3_out[h].opt()])

                def att_phase(w, hs, As, fillers=None):
                    """Software-pipelined rounds; norms at the end.
                    fillers[p] emits extra PE work after round p's AVs."""
                    Ss = score_mms(w, hs, 0)
                    Es = exp_mms(Ss, hs)
                    for p in range(NP):
                        if p + 1 < NP:
                            Sn = score_mms(w, hs, p + 1)
                            En = exp_mms(Sn, hs)
                        av_mms(hs, p, Es, As)
                        if fillers is not None and p in fillers:
                            fillers[p]()
                        if p + 1 < NP:
                            Es = En
                    for h in hs:
                        norm_store(w, h, As[h])

                # -- proj(w) pieces for full-window gathers (w = 0..2) --
                proj_st = {}

                def proj_dma(w):
                    ao = AS.tile([128, KP, FW], BF16, tag="ao", bufs=2,
                                 name="ao")
                    for kp in range(KP):
                        nc.sync.dma_start(
                            out=ao[:, kp, :],
                            in_=ag_out[w][kp * 128:(kp + 1) * 128, :])
                    proj_st[w] = ao

                def proj_m(w, mi):
                    ao = proj_st[w]
                    mlo, mhi = ((0, 128), (128, CHL))[mi]
                    pr = PS.tile([mhi - mlo, FW], F32, tag="mm", bufs=2,
                                 padded_shape=[128, FW], name="pr")
                    for kp in range(KP):
                        nc.tensor.matmul(
                            pr[:], lhsT=wpz_sb[:, kp, mlo:mhi],
                            rhs=ao[:, kp, :], start=(kp == 0), stop=False)
                    nc.tensor.matmul(
                        pr[:], lhsT=bp_sb[:, mlo:mhi], rhs=ones_q[:],
                        start=False, stop=True)
                    o_t = AS.tile([mhi - mlo, FW], F32, tag="o", bufs=2,
                                  padded_shape=[128, FW], name="o_t")
                    nc.vector.tensor_copy(o_t[:], pr[:])
                    nc.sync.dma_start(
                        out=out_d[mlo:mhi, w * FW:(w + 1) * FW], in_=o_t[:])

                def proj_last():
                    """proj(LW): three per-head gathered parts; only the
                    h=0 part (last gather) is in the true kernel tail."""
                    aos = []
                    for h in (2, 1, 0):
                        ao = AS.tile([128, 2, FW], BF16, tag="ao3", bufs=3,
                                     name="ao3")
                        for kp in range(2):
                            nc.sync.dma_start(
                                out=ao[:, kp, :],
                                in_=ag3_out[h][kp * 128:(kp + 1) * 128, :])
                        aos.append((h, ao))
                    for mlo, mhi in ((0, 128), (128, CHL)):
                        pr = PS.tile([mhi - mlo, FW], F32, tag="mm", bufs=2,
                                     padded_shape=[128, FW], name="pr3")
                        first = True
                        for h, ao in aos:
                            for kp in range(2):
                                nc.tensor.matmul(
                                    pr[:], lhsT=wps_sb[:, h, kp, mlo:mhi],
                                    rhs=ao[:, kp, :],
                                    start=first, stop=False)
                                first = False
                        nc.tensor.matmul(
                            pr[:], lhsT=bp_sb[:, mlo:mhi], rhs=ones_q[:],
                            start=False, stop=True)
                        o_t = AS.tile([mhi - mlo, FW], F32, tag="o", bufs=2,
                                      padded_shape=[128, FW], name="o_t3")
                        nc.vector.tensor_copy(o_t[:], pr[:])
                        nc.sync.dma_start(
                            out=out_d[mlo:mhi, LW * FW:(LW + 1) * FW],
                            in_=o_t[:])

                # ----------------- emission schedule -----------------
                # Window 0: h2+h1 rounds software-pipelined and interleaved
                # with QKV chunk groups (pair p needs k block p//2 and v
                # chunks 2p, 2p+1); then h0 solo; one gather at window end.
                A2, A1 = new_A(), new_A()
                emit_qk(0)
                emit_v(0); emit_v(1)
                w0_fill = {
                    0: lambda: (emit_qk(1), emit_v(2), emit_v(3)),
                    1: lambda: (emit_v(4), emit_v(5)),
                    2: lambda: (emit_qk(2), emit_v(6), emit_v(7)),
                    3: lambda: (emit_v(8), emit_v(9)),
                    4: lambda: (emit_qk(3), emit_v(10), emit_v(11)),
                    5: lambda: (emit_v(12), emit_v(13)),
                    6: lambda: (emit_v(14), emit_v(15)),
                }
                Ss = score_mms(0, [2, 1], 0)
                Es = exp_mms(Ss, [2, 1])
                for p in range(NP):
                    if p in w0_fill:
                        w0_fill[p]()
                    if p + 1 < NP:
                        Sn = score_mms(0, [2, 1], p + 1)
                        En = exp_mms(Sn, [2, 1])
                    av_mms([2, 1], p, Es, {2: A2, 1: A1})
                    if p + 1 < NP:
                        Es = En
                norm_store(0, 2, A2)
                norm_store(0, 1, A1)
                A0 = new_A()
                att_phase(0, [0], {0: A0})
                emit_gather(0)

                # Windows 1, 2: h2 solo, then h0/h1 paired with proj(w-1)
                # fillers; single gather at window end.
                for w in (1, 2):
                    A2 = new_A()
                    att_phase(w, [2], {2: A2})
                    A0, A1 = new_A(), new_A()
                    att_phase(w, [0, 1], {0: A0, 1: A1},
                              fillers={1: (lambda wp=w - 1: proj_dma(wp)),
                                       3: (lambda wp=w - 1: proj_m(wp, 0)),
                                       5: (lambda wp=w - 1: proj_m(wp, 1))})
                    emit_gather(w)

                # Window 3: three sequential head phases, per-head 64-row
                # gathers; proj(2) fillers spread over the h1/h0 phases.
                A2 = new_A()
                att_phase(LW, [2], {2: A2})
                emit_gather3(2)
                A1 = new_A()
                att_phase(LW, [1], {1: A1},
                          fillers={2: lambda: proj_dma(2),
                                   4: lambda: proj_m(2, 0)})
                emit_gather3(1)
                A0 = new_A()
                att_phase(LW, [0], {0: A0},
                          fillers={2: lambda: proj_m(2, 1)})
                emit_gather3(0)
                proj_last()
    nc.finalize()
    return nc


def get_nc():
    if "nc" not in _CACHE:
        _CACHE["nc"] = _build_nc()
    return _CACHE["nc"]


def _pack128(a):
    """[K, M] -> [128, K//128, M] partition-major packing."""
    from ml_dtypes import bfloat16

    k, m = a.shape
    return np.ascontiguousarray(
        a.reshape(k // 128, 128, m).transpose(1, 0, 2)).astype(bfloat16)


def make_in_maps(x, w_qkv, w_proj, b_proj):
    from ml_dtypes import bfloat16

    x = np.asarray(x, dtype=np.float32)
    w_qkv = np.asarray(w_qkv, dtype=np.float32)
    w_proj = np.asarray(w_proj, dtype=np.float32)
    b_proj = np.asarray(b_proj, dtype=np.float32)
    in_maps = []
    for core in range(NCORES):
        b, g = divmod(core, G)
        cs = slice(g * CHL, (g + 1) * CHL)
        wp = w_proj[:, cs]                      # [768, 192] natural rows
        # per-head split for window 3: partition p of (h, kp) chunk is
        # w_proj row 192*(2*kp) + 64*h + p for p < 64, rank 2*kp+1 above.
        wps = np.empty((128, HL, 2, CHL), np.float32)
        for h in range(HL):
            for kp in range(2):
                r0, r1 = 2 * kp, 2 * kp + 1
                wps[0:64, h, kp, :] = wp[192 * r0 + 64 * h:
                                         192 * r0 + 64 * h + 64]
                wps[64:128, h, kp, :] = wp[192 * r1 + 64 * h:
                                           192 * r1 + 64 * h + 64]
        im = {
            "xT": _pack128(np.ascontiguousarray(x[b].T)),
            "wq": _pack128(np.ascontiguousarray(w_qkv[:, 0 * C:1 * C][:, cs])),
            "wk": _pack128(np.ascontiguousarray(w_qkv[:, 1 * C:2 * C][:, cs])),
            "wv": _pack128(np.ascontiguousarray(w_qkv[:, 2 * C:3 * C][:, cs])),
            "wpz": _pack128(np.ascontiguousarray(wp)),
            "wps": wps.astype(bfloat16),
            "bp": np.ascontiguousarray(
                b_proj[cs].reshape(1, CHL)).astype(bfloat16),
        }
        in_maps.append(im)
    return in_maps


def unshard(results):
    out = np.empty((B, N, C), dtype=np.float32)
    for b in range(B):
        outT = np.concatenate(
            [results[b * G + g]["out"] for g in range(G)], axis=0)
        out[b] = outT.T
    return out


def kernel(x, w_qkv, w_proj, b_proj):
    from concourse.bass_utils import run_bass_kernel_spmd

    nc = get_nc()
    in_maps = make_in_maps(x, w_qkv, w_proj, b_proj)
    res = run_bass_kernel_spmd(nc, in_maps, list(range(NCORES)))
    return unshard(res.results)
